# revision 46
# baseline (speedup 1.0000x reference)
"""CLIP-style contrastive (NT-Xent) loss on 8 Trainium2 NeuronCores.

Strategy (data-parallel, per sharding hint):
  - Shard the batch (4096) across 8 cores: 512 rows of x_image/x_text each.
  - Each core projects its shard through both towers in TRANSPOSED
    activation layout ([feat_partitions, batch_free]); all operands are
    host-packed into [128, wide] bf16 tensors so every DMA moves >=2KB
    contiguous per partition, and the encoder k-chunks stream through a
    rotating SBUF pool so matmuls start as soon as chunk 0 lands.
  - The IMAGE tower is projected + L2-normalized first and its [128, 512]
    bf16 projections AllGathered immediately, overlapping the TEXT tower's
    compute; the text AllGather follows on the same CC stream.  The sim
    loop consumes image columns first, so the 64us ScalarE exp tail starts
    as soon as AG1 lands and completely hides AG2.
  - L2-normalize uses DVE reciprocal + ScalarE Sqrt (rsqrt without the
    Ln/Exp activation-table reloads; the single Exp table load for the sim
    loop hides in the AG wait).
  - Each core computes its 1024 rows of the global 8192x8192 similarity
    matrix in [128, 2048] PSUM chunks (bf16 matmuls, fp32 accumulate),
    applies exp(sim/t) on ScalarE with fused per-row accumulation
    (accum_out).
  - Device returns raw per-(row,chunk) partial sums [128, 32] plus
    pos/diag rows [3, 512]; host reduces and finishes in fp64:
        T'_r   = T_r - exp(diag_r/t) + exp(pos_r/t)
        loss_r = log(T'_r) - pos_r/t
    (pos/diag are computed from the bf16-rounded projections exactly like
    the similarity matmul computes those entries, so the big cancellation
    in T' is between nearly identical quantities.)
"""

import numpy as np
import ml_dtypes

import concourse.bacc as bacc
import concourse.bass as bass
import concourse.mybir as mybir
import concourse.tile as tile
from concourse.bass_utils import run_bass_kernel_spmd

NCORES = 8
B, DIN, DE, DH, DP = 4096, 1024, 512, 256, 128
S = B // NCORES            # 512: per-core batch shard
ROWS = 2 * S               # 1024 sim rows owned per core (z1 + z2 shard)
N = 2 * B                  # 8192 global rows
TEMP = 0.07
INV_T = 1.0 / TEMP
KE = DIN // 128            # 8 encoder contraction chunks

F32 = mybir.dt.float32
BF16 = mybir.dt.bfloat16
FP8 = mybir.dt.float8e4
NPBF = ml_dtypes.bfloat16

_CACHE: dict = {}


def _build():
    nc = bacc.Bacc("TRN2", target_bir_lowering=False, debug=False,
                   num_devices=NCORES)

    t_in = {}
    for m in ("img", "txt"):
        # per k-chunk: [We_k (512) | xT_k (512)] -> 2KB/partition contiguous
        t_in[f"enc_{m}"] = nc.dram_tensor(f"enc_{m}", [128, KE * 1024], BF16,
                                          kind="ExternalInput")
    # [wp1_img (1024) | wp1_txt (1024) | wp2_img (256) | wp2_txt (256)]
    t_in["wp"] = nc.dram_tensor("wp", [128, 2560], BF16, kind="ExternalInput")
    # [beT_i(4) bp1T_i(2) bp2T_i(1) beT_t(4) bp1T_t(2) bp2T_t(1)]
    t_in["biasT"] = nc.dram_tensor("biasT", [128, 14], F32,
                                   kind="ExternalInput")
    out_stats = nc.dram_tensor("stats", [128, 40], F32, kind="ExternalOutput")
    out_rows = nc.dram_tensor("rows", [4, S], F32, kind="ExternalOutput")

    with tile.TileContext(nc) as tc:
        _emit(nc, tc, t_in, out_stats, out_rows)
    nc.compile()
    return nc


def _emit(nc, tc, t_in, out_stats, out_rows):
    Exp = mybir.ActivationFunctionType.Exp
    Sqrt = mybir.ActivationFunctionType.Sqrt
    add = mybir.AluOpType.add
    mx = mybir.AluOpType.max

    NCHUNK = 2048                  # columns per PSUM super-chunk (4 banks)
    NTT = N // NCHUNK              # 4
    NRC = ROWS // 128              # 8 row chunks

    with tc.tile_pool(name="const", bufs=1) as cpool, \
         tc.tile_pool(name="encp", bufs=4) as encp, \
         tc.tile_pool(name="wpool", bufs=1) as wpool, \
         tc.tile_pool(name="hg", bufs=2) as hgp, \
         tc.tile_pool(name="actpool", bufs=1) as apool, \
         tc.tile_pool(name="rowsb", bufs=3) as rsb, \
         tc.tile_pool(name="psum", bufs=2, space="PSUM") as pps, \
         tc.tile_pool(name="escp", bufs=2) as escp, \
         tc.tile_pool(name="dram", bufs=1, space="DRAM") as dram:

        ones_col = cpool.tile([128, 1], F32)
        nc.any.memset(ones_col[:], 1.0)
        ones_row = cpool.tile([1, 128], F32)
        nc.any.memset(ones_row[:], 1.0)

        # ---- weight/bias/x DMAs: few, wide, two queues ----
        biasT = wpool.tile([128, 14], F32)
        nc.sync.dma_start(out=biasT[:], in_=t_in["biasT"][:, :])
        wp = wpool.tile([128, 2560], BF16)
        nc.scalar.dma_start(out=wp[:], in_=t_in["wp"][:, :])

        enc_tiles = {}
        for m in ("img", "txt"):
            for k in range(KE):
                tl = encp.tile([128, 1024], BF16, tag="enc")
                q = nc.sync if (k % 2 == 0) else nc.scalar
                q.dma_start(out=tl[:], in_=t_in[f"enc_{m}"]
                            [:, 1024 * k:1024 * (k + 1)])
                enc_tiles[(m, k)] = tl

        # ---- per-tower: project, normalize, AllGather right away ----
        znb, znb8b, cc_out = {}, {}, {}
        for mi, m in enumerate(("img", "txt")):
            boff = 7 * mi  # bias column offset for this tower
            h_ps = pps.tile([128, 4 * S], F32, tag="simps")
            for k in range(KE):
                tl = enc_tiles[(m, k)]
                for mm in range(4):
                    nc.tensor.matmul(
                        h_ps[:, S * mm:S * (mm + 1)],
                        tl[:, 128 * mm:128 * (mm + 1)],
                        tl[:, 512:1024],
                        start=(k == 0), stop=(k == KE - 1))
            h = hgp.tile([128, 4 * S], BF16, tag="h")
            for mm in range(4):
                nc.vector.tensor_scalar(
                    out=h[:, mm * S:(mm + 1) * S],
                    in0=h_ps[:, mm * S:(mm + 1) * S],
                    scalar1=biasT[:, boff + mm:boff + mm + 1],
                    scalar2=None, op0=add)
            g_ps = pps.tile([128, 2 * S], F32, tag="simps")
            for k2 in range(4):
                for mm2 in range(2):
                    nc.tensor.matmul(
                        g_ps[:, S * mm2:S * (mm2 + 1)],
                        wp[:, 1024 * mi + 256 * k2 + 128 * mm2:
                           1024 * mi + 256 * k2 + 128 * (mm2 + 1)],
                        h[:, S * k2:S * (k2 + 1)],
                        start=(k2 == 0), stop=(k2 == 3))
            g = hgp.tile([128, 2 * S], BF16, tag="g")
            for mm2 in range(2):
                nc.vector.tensor_scalar(
                    out=g[:, mm2 * S:(mm2 + 1) * S],
                    in0=g_ps[:, mm2 * S:(mm2 + 1) * S],
                    scalar1=biasT[:, boff + 4 + mm2:boff + 5 + mm2],
                    scalar2=0.0, op0=add, op1=mx)
            z_ps = pps.tile([128, S], F32, tag="simps")
            for k3 in range(2):
                nc.tensor.matmul(
                    z_ps[:],
                    wp[:, 2048 + 256 * mi + 128 * k3:
                       2048 + 256 * mi + 128 * (k3 + 1)],
                    g[:, S * k3:S * (k3 + 1)],
                    start=(k3 == 0), stop=(k3 == 1))
            z = apool.tile([128, S], F32, name=f"z_{m}")
            nc.vector.tensor_scalar(
                out=z[:], in0=z_ps[:],
                scalar1=biasT[:, boff + 6:boff + 7], scalar2=None, op0=add)

            # rsqrt normalize: DVE reciprocal + ScalarE Sqrt (no table churn)
            sq = rsb.tile([128, S], F32, tag="sq")
            nc.vector.tensor_mul(sq[:], z[:], z[:])
            pssq = pps.tile([1, S], F32, tag="simps")
            nc.tensor.matmul(pssq[:], ones_col[:], sq[:], start=True,
                             stop=True)
            rec = rsb.tile([1, S], F32, tag="rec")
            nc.vector.reciprocal(rec[:], pssq[:])
            inv = rsb.tile([1, S], F32, tag="inv")
            nc.scalar.activation(inv[:], rec[:], Sqrt)
            pinvb = pps.tile([128, S], F32, tag="simps")
            nc.tensor.matmul(pinvb[:], ones_row[:], inv[:], start=True,
                             stop=True)
            zb = apool.tile([128, S], BF16, name=f"znb_{m}")
            nc.vector.tensor_mul(zb[:], z[:], pinvb[:])
            znb[m] = zb

            # AllGather this tower immediately (img AG overlaps txt tower).
            # The payload is fp8e4: halves the collective bytes.  fp8 values
            # are exactly representable in bf16, so znb8b (the local
            # roundtrip) lets pos/diag be computed exactly as the mixed
            # bf16 x fp8 sim matmul computes those entries.
            z8 = apool.tile([128, S], FP8, name=f"znb8_{m}")
            nc.vector.tensor_copy(z8[:], zb[:])
            z8b = apool.tile([128, S], BF16, name=f"znb8b_{m}")
            nc.vector.tensor_copy(z8b[:], z8[:])
            znb8b[m] = z8b
            cc_in = dram.tile([128, S], FP8, name=f"cc_in_{m}")
            nc.scalar.dma_start(out=cc_in[:, :], in_=z8[:])
            cc_o = dram.tile([128 * NCORES, S], FP8, name=f"cc_out_{m}",
                             addr_space="Shared")
            nc.gpsimd.collective_compute(
                "AllGather", mybir.AluOpType.bypass,
                replica_groups=[list(range(NCORES))],
                ins=[cc_in[:]], outs=[cc_o[:]])
            cc_out[m] = cc_o

        # hoist the Exp activation-table load into the idle AG-wait window
        # (otherwise it lands right before the first sim exp, on the ramp)
        dmy = rsb.tile([1, 1], F32, tag="dmy")
        nc.scalar.activation(dmy[:], inv[:, 0:1], Exp)

        # ---- pos / self-diag rows, exactly as the sim matmul computes
        #      them: bf16 row x fp8-roundtripped column (overlap AGs) ----
        for r, (a, b) in enumerate((("img", "txt"), ("txt", "img"),
                                    ("img", "img"), ("txt", "txt"))):
            prod = rsb.tile([128, S], F32, tag="prod")
            nc.vector.tensor_mul(prod[:], znb[a][:], znb8b[b][:])
            pr = pps.tile([1, S], F32, tag="simps")
            nc.tensor.matmul(pr[:], ones_col[:], prod[:], start=True,
                             stop=True)
            row_sb = rsb.tile([1, S], F32, tag="rowsb")
            nc.vector.tensor_copy(row_sb[:], pr[:])
            nc.sync.dma_start(out=out_rows[r:r + 1, :], in_=row_sb[:])

        # ---- load gathered projections: 4 independent [128, 2048] tiles,
        #      img tiles first so the sim loop can start after AG1 ----
        zf8 = [apool.tile([128, NCHUNK], FP8, name=f"zf8{t}")
               for t in range(NTT)]
        zf = [apool.tile([128, NCHUNK], BF16, name=f"zf{t}")
              for t in range(NTT)]
        for mi, m in enumerate(("img", "txt")):
            for j in range(NCORES):
                # txt loads are AG2-gated: keep them OFF the scalar queue so
                # the ScalarE stream (sim exp on img columns) never waits on
                # AG2 before it has to.
                q = nc.scalar if (mi == 0 and j % 2 == 1) else nc.sync
                ti, sl = 2 * mi + j // 4, (j % 4) * S
                q.dma_start(
                    out=zf8[ti][:, sl:sl + S],
                    in_=cc_out[m][128 * j:128 * (j + 1), :])
                # widen to bf16 on the (idle) DVE so the sim matmuls run at
                # full bf16 x bf16 speed; fp8 -> bf16 is exact
                nc.vector.tensor_copy(zf[ti][:, sl:sl + S],
                                      zf8[ti][:, sl:sl + S])

        # ---- main loop: sim rows + exp + fused row sums ----
        # the first two groups are 1024 wide so the first exp only needs
        # two gather loads + two matmuls after AG1 lands
        groups = [(0, 0, 1024), (0, 1024, 1024), (1, 0, 2048),
                  (2, 0, 2048), (3, 0, 2048)]
        NGR = len(groups)
        stats = apool.tile([128, NRC * NGR], F32)
        for tt, (zi, c0, w) in enumerate(groups):
            for rc in range(NRC):
                if rc < 4:
                    lhs = znb["img"][:, 128 * rc:128 * (rc + 1)]
                else:
                    lhs = znb["txt"][:, 128 * (rc - 4):128 * (rc - 3)]
                ps = pps.tile([128, w], F32, tag="simps")
                for q in range(w // 512):
                    nc.tensor.matmul(
                        ps[:, 512 * q:512 * (q + 1)], lhs,
                        zf[zi][:, c0 + 512 * q:c0 + 512 * (q + 1)],
                        start=True, stop=True)
                esc = escp.tile([128, w], BF16, tag="esc")
                nc.scalar.activation(
                    esc[:], ps[:], Exp, scale=INV_T,
                    accum_out=stats[:, NGR * rc + tt: NGR * rc + tt + 1])

        nc.sync.dma_start(out=out_stats[:, :], in_=stats[:])


def _prep_in_maps(inputs):
    f32 = np.float32
    host = {}
    # encoder+x chunks (x differs per core; weights shared)
    We = {m: np.asarray(inputs[f"We_{m}"], f32).reshape(KE, 128, DE)
          for m in ("img", "txt")}
    x = {"img": np.asarray(inputs["x_image"], f32),
         "txt": np.asarray(inputs["x_text"], f32)}

    wp_parts = []
    for m in ("img", "txt"):
        wp1 = np.asarray(inputs[f"Wp1_{m}"], f32).reshape(4, 128, DH)
        wp_parts.append(wp1.transpose(1, 0, 2).reshape(128, 4 * DH))
    for m in ("img", "txt"):
        wp2 = np.asarray(inputs[f"Wp2_{m}"], f32).reshape(2, 128, DP)
        wp_parts.append(wp2.transpose(1, 0, 2).reshape(128, 2 * DP))
    host["wp"] = np.ascontiguousarray(
        np.concatenate(wp_parts, axis=1)).astype(NPBF)

    bias_parts = []
    for m in ("img", "txt"):
        bias_parts.append(np.asarray(inputs[f"be_{m}"], f32)
                          .reshape(4, 128).T)
        bias_parts.append(np.asarray(inputs[f"bp1_{m}"], f32)
                          .reshape(2, 128).T)
        bias_parts.append(np.asarray(inputs[f"bp2_{m}"], f32)
                          .reshape(1, 128).T)
    host["biasT"] = np.ascontiguousarray(np.concatenate(bias_parts, axis=1))

    in_maps = []
    for c in range(NCORES):
        mp = dict(host)
        for m in ("img", "txt"):
            xT = np.ascontiguousarray(
                x[m][c * S:(c + 1) * S].T).reshape(KE, 128, S)
            enc = np.concatenate([We[m], xT], axis=2)       # (8,128,1024)
            mp[f"enc_{m}"] = np.ascontiguousarray(
                enc.transpose(1, 0, 2).reshape(128, KE * 1024)).astype(NPBF)
        in_maps.append(mp)
    return in_maps


def _finish_host(results):
    """Host-side fp64 finish: combine per-core stats/rows into the loss."""
    total = 0.0
    t = TEMP
    for c in range(NCORES):
        stats = np.asarray(results[c]["stats"], np.float64)  # [128, 32]
        rows = np.asarray(results[c]["rows"], np.float64)    # [4, 512]
        T = stats.reshape(128, 8, 5).sum(axis=2)             # [128, rc]
        for rc in range(8):
            k = rc % 4
            sl = slice(128 * k, 128 * (k + 1))
            dg = rows[2, sl] if rc < 4 else rows[3, sl]
            pos = rows[0, sl] if rc < 4 else rows[1, sl]
            Tp = T[:, rc] - np.exp(dg / t) + np.exp(pos / t)
            total += float(np.sum(np.log(Tp) - pos / t))
    return np.float32(total / N)


def kernel(**inputs) -> np.ndarray:
    nc = _CACHE.get("nc")
    if nc is None:
        nc = _build()
        _CACHE["nc"] = nc
    res = run_bass_kernel_spmd(nc, _prep_in_maps(inputs),
                               core_ids=list(range(NCORES)))
    return _finish_host(res.results)


# revision 47
# speedup vs baseline: 1.6951x; 1.6951x over previous
"""CLIP-style contrastive (NT-Xent) loss on 8 Trainium2 NeuronCores.

Strategy (data-parallel, per sharding hint):
  - Shard the batch (4096) across 8 cores: 512 rows of x_image/x_text each.
  - Each core projects its shard through both towers in TRANSPOSED
    activation layout ([feat_partitions, batch_free]); all operands are
    host-packed into [128, wide] bf16 tensors so every DMA moves >=2KB
    contiguous per partition, and the encoder k-chunks stream through a
    rotating SBUF pool so matmuls start as soon as chunk 0 lands.
  - The IMAGE tower is projected + L2-normalized first and its [128, 512]
    bf16 projections AllGathered immediately, overlapping the TEXT tower's
    compute; the text AllGather follows on the same CC stream.  The sim
    loop consumes image columns first, so the 64us ScalarE exp tail starts
    as soon as AG1 lands and completely hides AG2.
  - L2-normalize uses DVE reciprocal + ScalarE Sqrt (rsqrt without the
    Ln/Exp activation-table reloads; the single Exp table load for the sim
    loop hides in the AG wait).
  - Each core computes its 1024 rows of the global 8192x8192 similarity
    matrix in [128, 2048] PSUM chunks (bf16 matmuls, fp32 accumulate),
    applies exp(sim/t) on ScalarE with fused per-row accumulation
    (accum_out).
  - Device returns raw per-(row,chunk) partial sums [128, 32] plus
    pos/diag rows [3, 512]; host reduces and finishes in fp64:
        T'_r   = T_r - exp(diag_r/t) + exp(pos_r/t)
        loss_r = log(T'_r) - pos_r/t
    (pos/diag are computed from the bf16-rounded projections exactly like
    the similarity matmul computes those entries, so the big cancellation
    in T' is between nearly identical quantities.)
"""

import numpy as np
import ml_dtypes

import concourse.bacc as bacc
import concourse.bass as bass
import concourse.mybir as mybir
import concourse.tile as tile
from concourse.bass_utils import run_bass_kernel_spmd

NCORES = 8
B, DIN, DE, DH, DP = 4096, 1024, 512, 256, 128
S = B // NCORES            # 512: per-core batch shard
ROWS = 2 * S               # 1024 sim rows owned per core (z1 + z2 shard)
N = 2 * B                  # 8192 global rows
TEMP = 0.07
INV_T = 1.0 / TEMP
KE = DIN // 128            # 8 encoder contraction chunks

F32 = mybir.dt.float32
BF16 = mybir.dt.bfloat16
NPBF = ml_dtypes.bfloat16

_CACHE: dict = {}


def _build():
    nc = bacc.Bacc("TRN2", target_bir_lowering=False, debug=False,
                   num_devices=NCORES)

    t_in = {}
    for m in ("img", "txt"):
        # per k-chunk: [We_k (512) | xT_k (512)] -> 2KB/partition contiguous
        t_in[f"enc_{m}"] = nc.dram_tensor(f"enc_{m}", [128, KE * 1024], BF16,
                                          kind="ExternalInput")
    # [wp1_img (1024) | wp1_txt (1024) | wp2_img (256) | wp2_txt (256)]
    t_in["wp"] = nc.dram_tensor("wp", [128, 2560], BF16, kind="ExternalInput")
    # [beT_i(4) bp1T_i(2) bp2T_i(1) beT_t(4) bp1T_t(2) bp2T_t(1)]
    t_in["biasT"] = nc.dram_tensor("biasT", [128, 14], F32,
                                   kind="ExternalInput")
    out_stats = nc.dram_tensor("stats", [128, 40], F32, kind="ExternalOutput")
    out_rows = nc.dram_tensor("rows", [3, S], F32, kind="ExternalOutput")

    with tile.TileContext(nc) as tc:
        _emit(nc, tc, t_in, out_stats, out_rows)
    nc.compile()
    return nc


def _emit(nc, tc, t_in, out_stats, out_rows):
    Exp = mybir.ActivationFunctionType.Exp
    Sqrt = mybir.ActivationFunctionType.Sqrt
    add = mybir.AluOpType.add
    mx = mybir.AluOpType.max

    NCHUNK = 2048                  # columns per PSUM super-chunk (4 banks)
    NTT = N // NCHUNK              # 4
    NRC = ROWS // 128              # 8 row chunks

    with tc.tile_pool(name="const", bufs=1) as cpool, \
         tc.tile_pool(name="encp", bufs=4) as encp, \
         tc.tile_pool(name="wpool", bufs=1) as wpool, \
         tc.tile_pool(name="hg", bufs=2) as hgp, \
         tc.tile_pool(name="actpool", bufs=1) as apool, \
         tc.tile_pool(name="rowsb", bufs=3) as rsb, \
         tc.tile_pool(name="psum", bufs=2, space="PSUM") as pps, \
         tc.tile_pool(name="escp", bufs=2) as escp, \
         tc.tile_pool(name="dram", bufs=1, space="DRAM") as dram:

        ones_col = cpool.tile([128, 1], F32)
        nc.any.memset(ones_col[:], 1.0)
        ones_row = cpool.tile([1, 128], F32)
        nc.any.memset(ones_row[:], 1.0)

        # ---- weight/bias/x DMAs: few, wide, two queues ----
        biasT = wpool.tile([128, 14], F32)
        nc.sync.dma_start(out=biasT[:], in_=t_in["biasT"][:, :])
        wp = wpool.tile([128, 2560], BF16)
        nc.scalar.dma_start(out=wp[:], in_=t_in["wp"][:, :])

        enc_tiles = {}
        for m in ("img", "txt"):
            for k in range(KE):
                tl = encp.tile([128, 1024], BF16, tag="enc")
                q = nc.sync if (k % 2 == 0) else nc.scalar
                q.dma_start(out=tl[:], in_=t_in[f"enc_{m}"]
                            [:, 1024 * k:1024 * (k + 1)])
                enc_tiles[(m, k)] = tl

        # ---- per-tower: project, normalize, AllGather right away ----
        znb, cc_out = {}, {}
        for mi, m in enumerate(("img", "txt")):
            boff = 7 * mi  # bias column offset for this tower
            h_ps = pps.tile([128, 4 * S], F32, tag="simps")
            for k in range(KE):
                tl = enc_tiles[(m, k)]
                for mm in range(4):
                    nc.tensor.matmul(
                        h_ps[:, S * mm:S * (mm + 1)],
                        tl[:, 128 * mm:128 * (mm + 1)],
                        tl[:, 512:1024],
                        start=(k == 0), stop=(k == KE - 1))
            h = hgp.tile([128, 4 * S], BF16, tag="h")
            for mm in range(4):
                nc.vector.tensor_scalar(
                    out=h[:, mm * S:(mm + 1) * S],
                    in0=h_ps[:, mm * S:(mm + 1) * S],
                    scalar1=biasT[:, boff + mm:boff + mm + 1],
                    scalar2=None, op0=add)
            g_ps = pps.tile([128, 2 * S], F32, tag="simps")
            for k2 in range(4):
                for mm2 in range(2):
                    nc.tensor.matmul(
                        g_ps[:, S * mm2:S * (mm2 + 1)],
                        wp[:, 1024 * mi + 256 * k2 + 128 * mm2:
                           1024 * mi + 256 * k2 + 128 * (mm2 + 1)],
                        h[:, S * k2:S * (k2 + 1)],
                        start=(k2 == 0), stop=(k2 == 3))
            g = hgp.tile([128, 2 * S], BF16, tag="g")
            for mm2 in range(2):
                nc.vector.tensor_scalar(
                    out=g[:, mm2 * S:(mm2 + 1) * S],
                    in0=g_ps[:, mm2 * S:(mm2 + 1) * S],
                    scalar1=biasT[:, boff + 4 + mm2:boff + 5 + mm2],
                    scalar2=0.0, op0=add, op1=mx)
            z_ps = pps.tile([128, S], F32, tag="simps")
            for k3 in range(2):
                nc.tensor.matmul(
                    z_ps[:],
                    wp[:, 2048 + 256 * mi + 128 * k3:
                       2048 + 256 * mi + 128 * (k3 + 1)],
                    g[:, S * k3:S * (k3 + 1)],
                    start=(k3 == 0), stop=(k3 == 1))
            z = apool.tile([128, S], F32, name=f"z_{m}")
            nc.vector.tensor_scalar(
                out=z[:], in0=z_ps[:],
                scalar1=biasT[:, boff + 6:boff + 7], scalar2=None, op0=add)

            # rsqrt normalize: DVE reciprocal + ScalarE Sqrt (no table churn)
            sq = rsb.tile([128, S], F32, tag="sq")
            nc.vector.tensor_mul(sq[:], z[:], z[:])
            pssq = pps.tile([1, S], F32, tag="simps")
            nc.tensor.matmul(pssq[:], ones_col[:], sq[:], start=True,
                             stop=True)
            rec = rsb.tile([1, S], F32, tag="rec")
            nc.vector.reciprocal(rec[:], pssq[:])
            inv = rsb.tile([1, S], F32, tag="inv")
            nc.scalar.activation(inv[:], rec[:], Sqrt)
            pinvb = pps.tile([128, S], F32, tag="simps")
            nc.tensor.matmul(pinvb[:], ones_row[:], inv[:], start=True,
                             stop=True)
            zb = apool.tile([128, S], BF16, name=f"znb_{m}")
            nc.vector.tensor_mul(zb[:], z[:], pinvb[:])
            znb[m] = zb

            # AllGather this tower immediately (img AG overlaps txt tower)
            cc_in = dram.tile([128, S], BF16, name=f"cc_in_{m}")
            nc.scalar.dma_start(out=cc_in[:, :], in_=zb[:])
            cc_o = dram.tile([128 * NCORES, S], BF16, name=f"cc_out_{m}",
                             addr_space="Shared")
            nc.gpsimd.collective_compute(
                "AllGather", mybir.AluOpType.bypass,
                replica_groups=[list(range(NCORES))],
                ins=[cc_in[:]], outs=[cc_o[:]])
            cc_out[m] = cc_o

        # hoist the Exp activation-table load into the idle AG-wait window
        # (otherwise it lands right before the first sim exp, on the ramp)
        dmy = rsb.tile([1, 1], F32, tag="dmy")
        nc.scalar.activation(dmy[:], inv[:, 0:1], Exp)

        # ---- pos / self-diag rows from bf16 projections (overlap AGs) ----
        for r, (a, b) in enumerate((("img", "txt"), ("img", "img"),
                                    ("txt", "txt"))):
            prod = rsb.tile([128, S], F32, tag="prod")
            nc.vector.tensor_mul(prod[:], znb[a][:], znb[b][:])
            pr = pps.tile([1, S], F32, tag="simps")
            nc.tensor.matmul(pr[:], ones_col[:], prod[:], start=True,
                             stop=True)
            row_sb = rsb.tile([1, S], F32, tag="rowsb")
            nc.vector.tensor_copy(row_sb[:], pr[:])
            nc.sync.dma_start(out=out_rows[r:r + 1, :], in_=row_sb[:])

        # ---- load gathered projections: 4 independent [128, 2048] tiles,
        #      img tiles first so the sim loop can start after AG1 ----
        zf = [apool.tile([128, NCHUNK], BF16, name=f"zf{t}")
              for t in range(NTT)]
        for mi, m in enumerate(("img", "txt")):
            for j in range(NCORES):
                # txt loads are AG2-gated: keep them OFF the scalar queue so
                # the ScalarE stream (sim exp on img columns) never waits on
                # AG2 before it has to.
                q = nc.scalar if (mi == 0 and j % 2 == 1) else nc.sync
                q.dma_start(
                    out=zf[2 * mi + j // 4][:, (j % 4) * S:(j % 4 + 1) * S],
                    in_=cc_out[m][128 * j:128 * (j + 1), :])

        # ---- main loop: sim rows + exp + fused row sums ----
        # the first two groups are 1024 wide so the first exp only needs
        # two gather loads + two matmuls after AG1 lands
        groups = [(0, 0, 1024), (0, 1024, 1024), (1, 0, 2048),
                  (2, 0, 2048), (3, 0, 2048)]
        NGR = len(groups)
        stats = apool.tile([128, NRC * NGR], F32)
        for tt, (zi, c0, w) in enumerate(groups):
            for rc in range(NRC):
                if rc < 4:
                    lhs = znb["img"][:, 128 * rc:128 * (rc + 1)]
                else:
                    lhs = znb["txt"][:, 128 * (rc - 4):128 * (rc - 3)]
                ps = pps.tile([128, w], F32, tag="simps")
                for q in range(w // 512):
                    nc.tensor.matmul(
                        ps[:, 512 * q:512 * (q + 1)], lhs,
                        zf[zi][:, c0 + 512 * q:c0 + 512 * (q + 1)],
                        start=True, stop=True)
                esc = escp.tile([128, w], BF16, tag="esc")
                nc.scalar.activation(
                    esc[:], ps[:], Exp, scale=INV_T,
                    accum_out=stats[:, NGR * rc + tt: NGR * rc + tt + 1])

        nc.sync.dma_start(out=out_stats[:, :], in_=stats[:])


def _prep_in_maps(inputs):
    f32 = np.float32
    host = {}
    # encoder+x chunks (x differs per core; weights shared)
    We = {m: np.asarray(inputs[f"We_{m}"], f32).reshape(KE, 128, DE)
          for m in ("img", "txt")}
    x = {"img": np.asarray(inputs["x_image"], f32),
         "txt": np.asarray(inputs["x_text"], f32)}

    wp_parts = []
    for m in ("img", "txt"):
        wp1 = np.asarray(inputs[f"Wp1_{m}"], f32).reshape(4, 128, DH)
        wp_parts.append(wp1.transpose(1, 0, 2).reshape(128, 4 * DH))
    for m in ("img", "txt"):
        wp2 = np.asarray(inputs[f"Wp2_{m}"], f32).reshape(2, 128, DP)
        wp_parts.append(wp2.transpose(1, 0, 2).reshape(128, 2 * DP))
    host["wp"] = np.ascontiguousarray(
        np.concatenate(wp_parts, axis=1)).astype(NPBF)

    bias_parts = []
    for m in ("img", "txt"):
        bias_parts.append(np.asarray(inputs[f"be_{m}"], f32)
                          .reshape(4, 128).T)
        bias_parts.append(np.asarray(inputs[f"bp1_{m}"], f32)
                          .reshape(2, 128).T)
        bias_parts.append(np.asarray(inputs[f"bp2_{m}"], f32)
                          .reshape(1, 128).T)
    host["biasT"] = np.ascontiguousarray(np.concatenate(bias_parts, axis=1))

    in_maps = []
    for c in range(NCORES):
        mp = dict(host)
        for m in ("img", "txt"):
            xT = np.ascontiguousarray(
                x[m][c * S:(c + 1) * S].T).reshape(KE, 128, S)
            enc = np.concatenate([We[m], xT], axis=2)       # (8,128,1024)
            mp[f"enc_{m}"] = np.ascontiguousarray(
                enc.transpose(1, 0, 2).reshape(128, KE * 1024)).astype(NPBF)
        in_maps.append(mp)
    return in_maps


def _finish_host(results):
    """Host-side fp64 finish: combine per-core stats/rows into the loss."""
    total = 0.0
    t = TEMP
    for c in range(NCORES):
        stats = np.asarray(results[c]["stats"], np.float64)  # [128, 32]
        rows = np.asarray(results[c]["rows"], np.float64)    # [3, 512]
        T = stats.reshape(128, 8, 5).sum(axis=2)             # [128, rc]
        for rc in range(8):
            k = rc % 4
            sl = slice(128 * k, 128 * (k + 1))
            dg = rows[1, sl] if rc < 4 else rows[2, sl]
            pos = rows[0, sl]
            Tp = T[:, rc] - np.exp(dg / t) + np.exp(pos / t)
            total += float(np.sum(np.log(Tp) - pos / t))
    return np.float32(total / N)


def kernel(**inputs) -> np.ndarray:
    nc = _CACHE.get("nc")
    if nc is None:
        nc = _build()
        _CACHE["nc"] = nc
    res = run_bass_kernel_spmd(nc, _prep_in_maps(inputs),
                               core_ids=list(range(NCORES)))
    return _finish_host(res.results)
